# revision 10
# baseline (speedup 1.0000x reference)
"""Trainium2 Bass kernel for DBNN (double-exponential depthwise causal conv
+ zero-diag bilinear readout), square-readout formulation.

Math: conv kernel k[n,d] = omega_n*(1-exp(-d/tau_rise_n))*exp(-d/tau_decay_n)
   = omega_n*(a_n^d - b_n^d), a = exp(-1/tau_decay), b = exp(-1/tau_rise-1/tau_decay).
Let u = x (*) (a^d - b^d) (omega folded into the readout weights). Chained IIR:
    v[t]  = b*v[t-1] + x[t]
    u'[t] = a*u'[t-1] + v[t-1],  u = (a-b)*u'   ((a-b) folded into L)
Readout per (batch,t), G = diag(om) @ (zero_diag(W)+zero_diag(W)^T)/2 @ diag(om):
    out = u^T G u + omega^T u - 70
Eigendecompose G = S^T diag(sign) S (S = sqrt|lam| V^T). With c chosen so that
2 S^T (sign*c) = omega:
    out[t] = sum_m sign_m * (S u[:,t] + c)_m^2  - sum_m sign_m c_m^2 - 70
So the whole readout is: Q = L^T u' (PE, L[n,m] = S[m,n]*(a_n-b_n)),
shat = Square(Q + c) (Act engine, per-partition bias), then a +-1 one-hot
"sel" matmul reduces partitions (modes) into per-batch output rows (PE).
No elementwise multiplies remain on DVE/GpSimd - they only run the scans.

u'[0] would need a shifted operand; instead col 0 of u' is garbage and
out[:, 0] (= -70 exactly, since k[0]=0) is patched in the epilogue.

Sharding: data-parallel over batch B=32 across 8 cores (4 batches/core).
x is converted to fp16 on host (halves DMA); scan state stays fp32 internally.
"""

import numpy as np

import concourse.mybir as mybir
from concourse import bacc, bass
from concourse.tile import TileContext
from concourse.bass_utils import run_bass_kernel_spmd

F32 = mybir.dt.float32
F16 = mybir.dt.float16
BF16 = mybir.dt.bfloat16

B, N, T = 32, 256, 1024
NCORES = 8
NB = B // NCORES          # batches per core
NH = N // 128             # 128-partition halves of the channel dim
FB = 512                  # psum free-dim chunk (fp32 moving max / one bank)

# GpSimd does NOT support tensor_tensor_scan on TRN2 (ISA check rejects the
# scan opcode on Pool) -- all scans go on DVE.
SCAN_ENG = ["vector"] * 16
N_WARM = 12               # dummy matmuls to ramp the PE p-state while idle


def _emit(nc: bass.Bass):
    x_d = nc.dram_tensor("x", [NB, N, T], F16, kind="ExternalInput").ap()
    l_d = nc.dram_tensor("lmat", [N, N], F16, kind="ExternalInput").ap()
    c_d = nc.dram_tensor("consts", [N, 4], F32, kind="ExternalInput").ap()
    # sel[mh][p, b*NB+m] = sign[mh*128+p] if m == b else 0
    sel_d = nc.dram_tensor("sel", [128, NH * NB * NB], BF16,
                           kind="ExternalInput").ap()
    k_d = nc.dram_tensor("kv", [NB, 1], F32, kind="ExternalInput").ap()
    o_d = nc.dram_tensor("out", [NB, T], F32, kind="ExternalOutput").ap()

    mult = mybir.AluOpType.mult
    add = mybir.AluOpType.add
    Ident = mybir.ActivationFunctionType.Identity
    Square = mybir.ActivationFunctionType.Square

    def eng(name):
        return getattr(nc, name)

    with TileContext(nc) as tc:
        with (
            tc.tile_pool(name="cpool", bufs=1) as cpool,
            tc.tile_pool(name="xpool", bufs=3) as xpool,
            tc.tile_pool(name="vpool", bufs=3) as vpool,
            tc.tile_pool(name="upool", bufs=3) as upool,
            tc.tile_pool(name="spool", bufs=4) as spool,
            tc.tile_pool(name="qpool", bufs=2, space="PSUM") as qpool,
            tc.tile_pool(name="rpool", bufs=1, space="PSUM") as rpool,
            tc.tile_pool(name="opool", bufs=1) as opool,
        ):
            # --- constants: lt/sel via Act queue, ct/kv via DVE queue so the
            # SP queue can start streaming x immediately ---
            lt = []
            for h in range(NH):
                t_ = cpool.tile([128, N], F16, tag=f"l{h}")
                nc.scalar.dma_start(out=t_, in_=l_d[h * 128:(h + 1) * 128, :])
                lt.append(t_)
            sel = cpool.tile([128, NH * NB * NB], BF16, tag="sel")
            nc.scalar.dma_start(out=sel, in_=sel_d)
            ct = []
            for h in range(NH):
                t_ = cpool.tile([128, 4], F32, tag=f"c{h}")
                nc.gpsimd.dma_start(out=t_, in_=c_d[h * 128:(h + 1) * 128, :])
                ct.append(t_)
            kt = cpool.tile([NB, 1], F32, tag="kv")
            nc.gpsimd.dma_start(out=kt, in_=k_d)

            # Absorb const-DMA completions into the DVE/GPSIMD vector clocks
            # with one tiny op each so later scan instructions (wait-slot
            # limited) don't wait on the const DMA lanes directly.
            dummy = cpool.tile([128, 1], F32, tag="dummy")
            nc.vector.tensor_tensor(out=dummy[:, :], in0=ct[0][:, 3:4],
                                    in1=ct[1][:, 3:4], op=add)
            dummy2 = cpool.tile([128, 1], F32, tag="dummy2")
            nc.gpsimd.tensor_tensor(out=dummy2[:, :], in0=ct[0][:, 3:4],
                                    in1=ct[1][:, 3:4], op=add)

            # --- capability probe: strided scan output on DVE ---
            # (NOTE: Pool rejects ALL TensorScalarPtr ops incl. plain STT)
            probe2 = cpool.tile([128, 4], F32, tag="probe2")
            nc.vector.tensor_tensor_scan(
                probe2[:, 0:4:2], ct[0][:, 0:2], ct[1][:, 0:2], 0.0,
                mult, add)

            # --- PE p-state warmup: dummy matmuls while PE would sit idle
            # during the DMA/scan pipe fill (clock ramps 0.65->2.4 GHz over
            # ~3us of continuous busy) ---
            wsrc = cpool.tile([128, FB], F16, tag="wsrc")
            nc.gpsimd.memset(wsrc[:, :], 0.0)
            wq = rpool.tile([128, FB], F32, tag="warmq", space="PSUM")
            for _ in range(N_WARM):
                nc.tensor.matmul(wq[:, :], lhsT=wsrc[:, 0:128], rhs=wsrc[:, :],
                                 start=True, stop=True)

            a_bc = [ct[h][:, 0:1].broadcast_to([128, T]) for h in range(NH)]
            b_bc = [ct[h][:, 1:2].broadcast_to([128, T]) for h in range(NH)]

            # per-f accumulator tiles [NB, FB]: row b = output row for batch b
            rts = [rpool.tile([NB, FB], F32, tag=f"rt{f}", name=f"rt{f}")
                   for f in range(T // FB)]

            si = 0
            for b in range(NB):
                # --- conv: chained IIR scans ---
                uts = []
                for h in range(NH):
                    xt = xpool.tile([128, T], F16, tag=f"x{h}")
                    nc.sync.dma_start(out=xt, in_=x_d[b, h * 128:(h + 1) * 128, :])
                    vt = vpool.tile([128, T], F16, tag=f"v{h}")
                    eng(SCAN_ENG[si]).tensor_tensor_scan(
                        vt[:, :], b_bc[h], xt[:, :], 0.0, mult, add)
                    si += 1
                    ut = upool.tile([128, T], F16, tag=f"u{h}")
                    eng(SCAN_ENG[si]).tensor_tensor_scan(
                        ut[:, 1:T], a_bc[h][:, 0:T - 1], vt[:, 0:T - 1],
                        0.0, mult, add)
                    si += 1
                    uts.append(ut)

                # --- readout ---
                sts = []
                for mh in range(NH):
                    qt = qpool.tile([128, T], F32, tag="q")
                    for f in range(T // FB):
                        fs = slice(f * FB, (f + 1) * FB)
                        for nh in range(NH):
                            nc.tensor.matmul(
                                qt[:, fs],
                                lhsT=lt[nh][:, mh * 128:(mh + 1) * 128],
                                rhs=uts[nh][:, fs],
                                start=(nh == 0),
                                stop=(nh == NH - 1),
                            )
                    st_ = spool.tile([128, T], BF16, tag=f"s{mh}")
                    for f in range(T // FB):
                        fs = slice(f * FB, (f + 1) * FB)
                        nc.scalar.activation(st_[:, fs], qt[:, fs], Square,
                                             bias=ct[mh][:, 2:3])
                    sts.append(st_)

                for f in range(T // FB):
                    fs = slice(f * FB, (f + 1) * FB)
                    for mh in range(NH):
                        cs = mh * NB * NB + b * NB
                        nc.tensor.matmul(
                            rts[f][:, :],
                            lhsT=sel[:, cs:cs + NB],
                            rhs=sts[mh][:, fs],
                            start=(b == 0 and mh == 0),
                            stop=(b == NB - 1 and mh == NH - 1),
                            skip_group_check=True,
                        )

            # epilogue: +K bias, patch col 0 (= -70 exactly), one DMA out
            ot = opool.tile([NB, T], F32, tag="o")
            for f in range(T // FB):
                fs = slice(f * FB, (f + 1) * FB)
                nc.scalar.activation(ot[:, fs], rts[f][:, :], Ident,
                                     bias=kt[:, 0:1])
            nc.scalar.activation(ot[:, 0:1], rts[0][:, 0:1],
                                 mybir.ActivationFunctionType.Copy,
                                 bias=-70.0, scale=0.0)
            nc.sync.dma_start(out=o_d[:, :], in_=ot[:, :])


_CACHE = {}


def _build():
    if "nc" not in _CACHE:
        nc = bacc.Bacc("TRN2", target_bir_lowering=False, debug=False,
                       num_devices=NCORES)
        _emit(nc)
        nc.finalize()
        _CACHE["nc"] = nc
    return _CACHE["nc"]


def _host_prep(x, tau_rise, tau_decay, omega, W):
    x16 = np.ascontiguousarray(np.asarray(x, dtype=np.float16))
    tr = np.asarray(tau_rise, dtype=np.float64)
    td = np.asarray(tau_decay, dtype=np.float64)
    om = np.asarray(omega, dtype=np.float64)
    a = np.exp(-1.0 / td)
    b = np.exp(-1.0 / tr - 1.0 / td)
    Wm = np.asarray(W, dtype=np.float64)[0].copy()
    np.fill_diagonal(Wm, 0.0)
    G = om[:, None] * om[None, :] * (Wm + Wm.T) / 2.0
    lam, V = np.linalg.eigh(G)
    S = np.sqrt(np.abs(lam))[:, None] * V.T          # [M, N]
    sign = np.where(lam >= 0, 1.0, -1.0)
    z = np.linalg.solve(S.T, om / 2.0)
    c = sign * z
    K = float(-np.sum(sign * c * c) - 70.0)

    L = (S.T * (a - b)[:, None]).astype(np.float16)  # [n, m]
    consts = np.zeros((N, 4), dtype=np.float32)
    consts[:, 0] = a
    consts[:, 1] = b
    consts[:, 2] = c
    consts[:, 3] = 1.0
    sel = np.zeros((128, NH * NB * NB), dtype=np.float32)
    for mh in range(NH):
        for b_ in range(NB):
            sel[:, mh * NB * NB + b_ * NB + b_] = sign[mh * 128:(mh + 1) * 128]
    sel = sel.astype(mybir.dt.np(mybir.dt.bfloat16))
    kv = np.full((NB, 1), K, dtype=np.float32)
    return x16, L, consts, sel, kv


def make_in_maps(x, tau_rise, tau_decay, omega, W):
    x16, L, consts, sel, kv = _host_prep(x, tau_rise, tau_decay, omega, W)
    return [
        {"x": x16[c * NB:(c + 1) * NB], "lmat": L, "consts": consts,
         "sel": sel, "kv": kv}
        for c in range(NCORES)
    ]


def run(inputs, trace=False):
    nc = _build()
    in_maps = make_in_maps(**inputs)
    res = run_bass_kernel_spmd(nc, in_maps, list(range(NCORES)), trace=trace)
    out = np.concatenate([r["out"] for r in res.results], axis=0)
    return out.astype(np.float32), res


def kernel(x, tau_rise, tau_decay, omega, W):
    out, _ = run(dict(x=x, tau_rise=tau_rise, tau_decay=tau_decay,
                      omega=omega, W=W))
    return out
